# revision 9
# baseline (speedup 1.0000x reference)
"""Trainium2 Bass kernel for nn_Agent2Agent_emb (gnn_message_passing).

Reference computes, for each batch b:
    edge[b,m,n,e] = pairwise features of (agent1[b,m], agent2[b,n])   (E=8)
    out[b,m,n,h]  = einsum("mne,he->mnh", edge, W) + bias             (H=128)

Every edge feature is bilinear in per-m and per-n quantities, so the whole
output factors exactly as a rank-5 product

    out[b,m,n,h] = sum_{k<5} P[b,m,k] * R[b,k,n,h]

with P built from agent1 rows and R built from agent2 rows, W and bias
(see _build_factors).  The device kernel is then a tiny-K matmul that
expands [5 x N1] x [5 x (N2*H)] per batch -- pure memory-bound output
streaming, which matches the target regime.

The matmul runs in fp16 (K=5, no error-compensation split needed: fp16's
10 mantissa bits give ~7e-4 relative error vs the fp32 reference), which
cuts the rhs load to a third of a bf16 hi/lo variant.  Operands are
replicated at SBUF partition bases 0 and 32 so each psum tile's two
512-wide matmuls run concurrently on separate PE row groups (the
replication costs only a second 0.33 MB HBM load).

The kernel is output-store bound: 16.78 MB of fp16 output per core vs the
~358 GB/s per-core HBM limit (~47 us).  All scheduling aims to start the
store stream early and keep it saturated: loads are split per column
chunk (each piece gets its own semaphore -- partial thresholds on a
shared semaphore race, because engines with no data for a piece fire
their sem increments immediately), the first output unit is stored in
psum-tile granules, and store units alternate between the sync and
gpsimd DMA queues.

Sharding: one batch element per NeuronCore (B == n_cores == 8); each core
writes its own [N1, N2*H] slab, gathered on host by np.stack.
"""

import numpy as np

B, N1, N2, D, E, H = 8, 256, 256, 7, 8, 128
XY_SCALE = 10.0
NCORES = 8
K = 5           # contraction dim on device
FDIM = N2 * H   # 32768, flattened (n, h) free dim

# device tiling
OCH = 4096      # sbuf output-staging chunk (per-partition elements)
PCH = 1024      # psum tile free size (2 fp32 banks)
MM = 512        # free dim per matmul

# The device computes and stores the output in fp16 (upcast to fp32 on the
# host).  Output rounding gives ~4.9e-4 relative error, far below the fp16
# matmul noise, and halves the HBM store traffic that bounds this kernel.
OUT_DT = "float16"

NCH = FDIM // OCH      # 8 column chunks
NO = 8                 # output-staging ring depth
TILES_PER_CHUNK = 2 * (OCH // PCH)  # 2 mc x 4 fi = 8
NTILES = NCH * TILES_PER_CHUNK      # 64
NMM = NTILES * (PCH // MM)          # 128
NUNITS = NCH * 2                    # 16 (j, mc) store units of [128, OCH]
QB = 32         # partition base of the second operand replica (PE row group 1)

# copy-engine assignment per psum tile: even tiles on ScalarE, odd on
# VectorE, so the first two tiles' copies run concurrently at ramp time
_ENG = ["s" if T % 2 == 0 else "v" for T in range(NTILES)]
_CV_PRE = [sum(1 for t in range(T + 1) if _ENG[t] == "v") for T in range(NTILES)]
_CS_PRE = [sum(1 for t in range(T + 1) if _ENG[t] == "s") for T in range(NTILES)]


def _build_factors(agent1, agent2, W, b):
    """Host-side rank-5 factorization, cast to fp16.

    Returns AT [B, K, N1] fp16 (matmul lhsT) and RR [B, K, FDIM] fp16
    (matmul rhs) with out = AT.T @ RR per batch.
    """
    a1_f32 = np.asarray(agent1)
    a2_f32 = np.asarray(agent2)
    a1 = a1_f32.astype(np.float64)
    a2 = a2_f32.astype(np.float64)
    Wd = np.asarray(W).astype(np.float64)
    bd = np.asarray(b).astype(np.float64)

    f1 = (~np.all(a1_f32 == 0, axis=-1)).astype(np.float64)  # [B,N1]
    f2 = (~np.all(a2_f32 == 0, axis=-1)).astype(np.float64)  # [B,N2]

    x1x, x1y, s1, c1 = a1[..., 0], a1[..., 1], a1[..., 3], a1[..., 4]
    x2x, x2y, v2, s2, c2 = a2[..., 0], a2[..., 1], a2[..., 2], a2[..., 3], a2[..., 4]

    # m-side basis P [B, N1, 5]
    P = np.stack(
        [
            f1 * c1,
            f1 * s1,
            -f1 * (c1 * x1x + s1 * x1y),
            f1 * (s1 * x1x - c1 * x1y),
            np.ones_like(f1),
        ],
        axis=-1,
    )

    # n-side basis g [B, N2]
    g1 = f2 * x2x
    g2 = f2 * x2y
    g3 = f2
    g4 = f2 * s2
    g5 = f2 * c2
    g6 = f2 * s2 * v2
    g7 = f2 * c2 * v2
    g8 = a2[..., 5]
    g9 = a2[..., 6]

    s = XY_SCALE
    W0, W1, W2, W3, W4, W5, W6, W7 = (Wd[:, e] for e in range(8))

    def outer(g, w):  # [B,N2] x [H] -> [B,N2,H]
        return g[..., None] * w[None, None, :]

    R1 = (
        outer(g1, W0) / s
        + outer(g2, W1) / s
        + outer(g4, W2)
        + outer(g5, W3)
        + outer(g6, W4)
        + outer(g7, W5)
    )
    R2 = (
        outer(g2, W0) / s
        - outer(g1, W1) / s
        - outer(g5, W2)
        + outer(g4, W3)
        - outer(g7, W4)
        + outer(g6, W5)
    )
    R3 = outer(g3, W0) / s
    R4 = outer(g3, W1) / s
    R5 = outer(g8, W6) + outer(g9, W7) + bd[None, None, :]
    R = np.stack([R1, R2, R3, R4, R5], axis=1)  # [B, 5, N2, H]

    AT = np.ascontiguousarray(P.transpose(0, 2, 1)).astype(np.float16)
    RR = R.reshape(B, K, FDIM).astype(np.float16)
    return AT, RR


def build_bass():
    import concourse.mybir as mybir
    from concourse import bacc
    from contextlib import ExitStack

    nc = bacc.Bacc()
    out_dt = getattr(mybir.dt, OUT_DT)
    atr = nc.dram_tensor("atr", [K, N1], mybir.dt.float16, kind="ExternalInput")
    rr = nc.dram_tensor("rr", [K, FDIM], mybir.dt.float16, kind="ExternalInput")
    out = nc.dram_tensor("out", [N1, FDIM], out_dt, kind="ExternalOutput")

    ctx = ExitStack()
    with ctx:
        at_sb = ctx.enter_context(
            nc.sbuf_tensor("at_sb", [QB + K, N1], mybir.dt.float16)
        )
        r_sb = ctx.enter_context(
            nc.sbuf_tensor("r_sb", [QB + K, FDIM], mybir.dt.float16)
        )
        ot_sb = [
            ctx.enter_context(nc.sbuf_tensor(f"ot_sb{i}", [128, OCH], out_dt))
            for i in range(NO)
        ]
        ps = [
            ctx.enter_context(
                nc.psum_tensor(f"ps{i}", [128, PCH], mybir.dt.float32)
            )
            for i in range(4)
        ]
        s_at = ctx.enter_context(nc.semaphore("s_at"))
        s_r0 = ctx.enter_context(nc.semaphore("s_r0"))
        s_r1 = ctx.enter_context(nc.semaphore("s_r1"))
        s_rc = [ctx.enter_context(nc.semaphore(f"s_rc{j}")) for j in range(1, NCH)]
        s_mm = ctx.enter_context(nc.semaphore("s_mm"))
        s_cv = ctx.enter_context(nc.semaphore("s_cv"))
        s_cs = ctx.enter_context(nc.semaphore("s_cs"))
        s_sts = [ctx.enter_context(nc.semaphore(f"s_st{i}")) for i in range(NO)]
        s_scr = ctx.enter_context(nc.semaphore("s_scr"))
        block = ctx.enter_context(nc.Block())

        def tile_info(T):
            j = T // TILES_PER_CHUNK
            mc = (T // (OCH // PCH)) % 2
            fi = T % (OCH // PCH)
            return j, mc, fi

        class WaitTracker:
            """Skip waits already implied by earlier waits on this engine."""

            def __init__(self, eng):
                self.eng = eng
                self.seen = {}

            def wait(self, sem, val):
                key = id(sem)
                if self.seen.get(key, -1) >= val:
                    return
                self.seen[key] = val
                self.eng.wait_ge(sem, val)

        def copy_body(eng, which, inc_sem):
            w = WaitTracker(eng)
            for T in range(NTILES):
                if _ENG[T] != which:
                    continue
                j, mc, fi = tile_info(T)
                O = T // (OCH // PCH)
                w.wait(s_mm, 2 * (T + 1))
                if O >= NO:
                    w.wait(s_sts[O % NO], 16 * (O // NO))
                dst = ot_sb[O % NO][:, fi * PCH : (fi + 1) * PCH]
                if which == "v":
                    eng.tensor_copy(dst, ps[T % 4][:]).then_inc(inc_sem, 1)
                else:
                    eng.copy(dst, ps[T % 4][:]).then_inc(inc_sem, 1)

        @block.scalar
        def _(scalar):
            # dummy op: trigger the lazy ACT table load at t~0 so the first
            # real copy isn't gated on it (the table DMA takes ~1.3us and
            # otherwise fires ~7us in, delaying the whole store stream).
            # Writes a cell of ot_sb[0] that this engine's tile-0 copy
            # overwrites before the first store waits on it.
            scalar.copy(ot_sb[0][0:1, 0:1], ot_sb[0][0:1, 0:1])
            # bulk rhs loads ride this engine's HWDGE ring: a separate FIFO
            # from the sync ring, so they never delay store data, and HWDGE
            # emission is cheap (SWDGE/gpsimd costs ~0.6us per dma to emit)
            for base in (0, QB):
                scalar.dma_start(
                    r_sb[base : base + K, PCH:OCH], rr[:, PCH:OCH]
                ).then_inc(s_r1, 16)
            for j in range(1, NCH):
                for base in (0, QB):
                    scalar.dma_start(
                        r_sb[base : base + K, j * OCH : (j + 1) * OCH],
                        rr[:, j * OCH : (j + 1) * OCH],
                    ).then_inc(s_rc[j - 1], 16)
            copy_body(scalar, "s", s_cs)

        @block.vector
        def _(vector):
            vector.tensor_copy(ot_sb[0][0:1, PCH : PCH + 1], ot_sb[0][0:1, PCH : PCH + 1])
            copy_body(vector, "v", s_cv)

        def unit_ready(w, S):
            T_last = 4 * S + 3
            w.wait(s_cv, _CV_PRE[T_last])
            w.wait(s_cs, _CS_PRE[T_last])

        def store_unit(eng, S):
            j = S // 2
            mc = S % 2
            rows = slice(mc * 128, (mc + 1) * 128)
            eng.dma_start(
                out[rows, j * OCH : (j + 1) * OCH], ot_sb[S % NO][:]
            ).then_inc(s_sts[S % NO], 16)

        @block.sync
        def _(sync):
            # tiny at + first-psum-tile rhs loads lead the store queue
            # (~25 KB: negligible FIFO delay); the bulk rhs load trickles
            # on the gpsimd queue.  Each piece is loaded twice: partition
            # bases 0 and QB (PE row groups 0/1).
            for base in (0, QB):
                sync.dma_start(at_sb[base : base + K, :], atr[:]).then_inc(s_at, 16)
            for base in (0, QB):
                sync.dma_start(
                    r_sb[base : base + K, :PCH], rr[:, :PCH]
                ).then_inc(s_r0, 16)
            w = WaitTracker(sync)
            # unit 0 in psum-tile granules so the store stream starts as
            # soon as tile 0 is copied; then whole units
            for fi in range(OCH // PCH):
                w.wait(s_cv, _CV_PRE[fi])
                w.wait(s_cs, _CS_PRE[fi])
                d = sync.dma_start(
                    out[0:128, fi * PCH : (fi + 1) * PCH],
                    ot_sb[0][:, fi * PCH : (fi + 1) * PCH],
                )
                if fi == OCH // PCH - 1:
                    d.then_inc(s_sts[0], 16)
                else:
                    d.then_inc(s_scr, 16)
            for S in range(1, NUNITS):
                unit_ready(w, S)
                store_unit(sync, S)

        @block.tensor
        def _(tensor):
            w = WaitTracker(tensor)
            for i in range(NMM):
                T = i // 2
                g = i % 2
                j, mc, fi = tile_info(T)
                if j == 0 and fi == 0:
                    w.wait(s_at, 32)
                    w.wait(s_r0, 32)
                elif j == 0:
                    w.wait(s_r1, 32)
                else:
                    w.wait(s_rc[j - 1], 32)
                if g == 0 and T >= 4:
                    Tp = T - 4
                    if _ENG[Tp] == "v":
                        w.wait(s_cv, _CV_PRE[Tp])
                    else:
                        w.wait(s_cs, _CS_PRE[Tp])
                base = QB * g
                lo = j * OCH + fi * PCH + g * MM
                tensor.matmul(
                    ps[T % 4][:, g * MM : (g + 1) * MM],
                    at_sb[base : base + K, mc * 128 : (mc + 1) * 128],
                    r_sb[base : base + K, lo : lo + MM],
                    start=True,
                    stop=True,
                ).then_inc(s_mm, 1)

    nc.compile()
    return nc


_NC_CACHE = None


def _get_nc():
    global _NC_CACHE
    if _NC_CACHE is None:
        _NC_CACHE = build_bass()
    return _NC_CACHE


def run(agent1, agent2, W, b, trace=False):
    from concourse.bass_utils import run_bass_kernel_spmd

    AT, RR = _build_factors(agent1, agent2, W, b)
    in_maps = [
        {"atr": np.ascontiguousarray(AT[c]), "rr": np.ascontiguousarray(RR[c])}
        for c in range(NCORES)
    ]
    res = run_bass_kernel_spmd(
        _get_nc(), in_maps, core_ids=list(range(NCORES)), trace=trace
    )
    out = np.stack(
        [
            np.asarray(res.results[c]["out"]).astype(np.float32).reshape(N1, N2, H)
            for c in range(NCORES)
        ]
    )
    return out, res


def kernel(agent1, agent2, W, b):
    out, _ = run(agent1, agent2, W, b, trace=False)
    return out
